# revision 3
# baseline (speedup 1.0000x reference)
"""Trainium2 Bass kernel for a single-layer batch-first GRU (PyTorch gate order).

Problem: noise (256, 2048, 10) -> GRU(10 -> 64) -> out (256, 2048, 64), f32.

Strategy: pure data parallel over batch across 8 NeuronCores (32 rows each).
Per core, gate-major layout (gates/hidden on SBUF partitions, batch on the
free dim):
  - input projections gi = W_ih @ x_t are bulk-matmul'ed into PSUM slots for
    32 timesteps at a time (PE, N=512 matmuls),
  - the serial recurrence then runs one step at a time:
      PE   : psum_rz[s] += W_hh_rz @ h        (accumulate onto gi_rz)
      ACT  : rz = sigmoid(psum_rz[s] + bias_rz)
      PE   : psum_nh = W_hh_n @ h
      DVE  : m = (psum_nh + b_hh_n) * r       (fused scalar_tensor_tensor)
      PE   : psum_gn[s] += I @ m              (identity matmul as PSUM add)
      ACT  : n = tanh(psum_gn[s] + b_ih_n)
      DVE  : p = z * h ; q = (z - 1) * n ; h' = p - q
  - h' is written straight into a (64, 32, 32) history tile that doubles as
    the DMA staging buffer; output DRAM is H-major (64, 2048, 32) per core and
    transposed back to (B, T, H) on the host.
"""

import numpy as np
from contextlib import ExitStack

import concourse.bass as bass
import concourse.tile as tile
from concourse import mybir
from concourse.bass_utils import run_bass_kernel_spmd

F32 = mybir.dt.float32
AF = mybir.ActivationFunctionType
OP = mybir.AluOpType

B, T, NI, NH = 256, 2048, 10, 64
NCORES = 8
BLOC = B // NCORES          # 32 batch rows per core
S = 32                      # timesteps whose gi live in PSUM at once
NQ = 4                      # noise staged into SBUF in quarters
QLEN = T // NQ              # 512 timesteps per quarter

TRACE = False               # test harness flips this for profiled runs
_LAST_RESULTS = {}          # stash for exec_time introspection by test.py


def _split_excess_waits(nc, cap=1):
    """walrus (CoreV3) rejects instructions carrying more than `cap` sem
    waits; hoist the excess onto same-engine Drain nops just before."""
    for f in nc.m.functions:
        for bb in f.blocks:
            new_insts = []
            for inst in bb.instructions:
                si = inst.sync_info
                if si and si.on_wait and len(si.on_wait) > cap:
                    waits = list(si.on_wait)
                    extra, keep = waits[:-cap], waits[-cap:]
                    for k, i in enumerate(range(0, len(extra), cap)):
                        nop = mybir.InstDrain(
                            name=f"{inst.name}_ws{k}", ins=[], outs=[]
                        )
                        nop.engine = inst.engine
                        nop.sync_info = mybir.SyncInfo(
                            on_wait=extra[i : i + cap], on_update=[]
                        )
                        new_insts.append(nop)
                    si.on_wait = keep
                new_insts.append(inst)
            bb.instructions = new_insts
    return nc


def _build():
    nc = bass.Bass("TRN2", target_bir_lowering=False, debug=False)

    noise_d = nc.declare_dram_parameter("noiseT", [NI, T, BLOC], F32, False)
    wihT_d = nc.declare_dram_parameter("w_ihT", [NI, 3 * NH], F32, False)
    whhT_d = nc.declare_dram_parameter("w_hhT", [NH, 3 * NH], F32, False)
    bias_rz_d = nc.declare_dram_parameter("bias_rz", [2 * NH, 1], F32, False)
    b_ihn_d = nc.declare_dram_parameter("b_ihn", [NH, 1], F32, False)
    b_hhn_d = nc.declare_dram_parameter("b_hhn", [2 * NH, 1], F32, False)
    ident_d = nc.declare_dram_parameter("ident", [2 * NH, NH], F32, False)
    out_d = nc.declare_dram_parameter("outT", [NH, T, BLOC], F32, True)

    with tile.TileContext(nc) as tc, ExitStack() as ctx:
        const = ctx.enter_context(tc.tile_pool(name="const", bufs=1))
        noisep = ctx.enter_context(tc.tile_pool(name="noise", bufs=2))
        work = ctx.enter_context(tc.tile_pool(name="work", bufs=3))
        hist = ctx.enter_context(tc.tile_pool(name="hist", bufs=1))
        psum_main = ctx.enter_context(
            tc.tile_pool(name="psum_main", bufs=1, space="PSUM")
        )
        psum_nh_pool = ctx.enter_context(
            tc.tile_pool(name="psum_nh", bufs=2, space="PSUM")
        )

        # --- constants -----------------------------------------------------
        wihT = const.tile([NI, 3 * NH], F32)
        nc.sync.dma_start(out=wihT, in_=wihT_d[:])
        whhT = const.tile([NH, 3 * NH], F32)
        nc.sync.dma_start(out=whhT, in_=whhT_d[:])
        bias_rz = const.tile([2 * NH, 1], F32)
        nc.sync.dma_start(out=bias_rz, in_=bias_rz_d[:])
        b_ihn = const.tile([NH, 1], F32)
        nc.sync.dma_start(out=b_ihn, in_=b_ihn_d[:])
        b_hhn = const.tile([2 * NH, 1], F32)
        nc.sync.dma_start(out=b_hhn, in_=b_hhn_d[:])
        ident = const.tile([2 * NH, NH], F32)
        nc.sync.dma_start(out=ident, in_=ident_d[:])

        # --- persistent state ---------------------------------------------
        # out_hist doubles as h-state carrier: slot s holds h_t of the s-th
        # step of the current chunk; slot S-1 enters each chunk holding the
        # previous chunk's final h.
        out_hist = hist.tile([NH, S, BLOC], F32)
        nc.vector.memset(out_hist[:, S - 1, :], 0.0)  # h_0 = 0

        psum_rz = psum_main.tile([2 * NH, S, BLOC], F32)  # 2 banks
        psum_gn = psum_main.tile([NH, S, BLOC], F32)      # 2 banks

        for q in range(NQ):
            with tc.For_i(0, QLEN, S) as iv:
                noise_sb = noisep.tile([NI, S, BLOC], F32, tag="noise")
                nc.sync.dma_start(
                    out=noise_sb, in_=noise_d[:, bass.ds(q * QLEN + iv, S), :]
                )
                # ---- bulk input projections for S steps into PSUM ----
                for j in range(S // 16):  # 16 steps x 32 batch = 512 free
                    rhs = noise_sb[:, j * 16 : (j + 1) * 16, :]
                    nc.tensor.matmul(
                        psum_rz[:, j * 16 : (j + 1) * 16, :],
                        wihT[:, 0 : 2 * NH],
                        rhs,
                        start=True,
                        stop=False,
                        skip_group_check=True,
                    )
                    nc.tensor.matmul(
                        psum_gn[:, j * 16 : (j + 1) * 16, :],
                        wihT[:, 2 * NH : 3 * NH],
                        rhs,
                        start=True,
                        stop=False,
                        skip_group_check=True,
                    )

                # ---- serial recurrence ----
                for s in range(S):
                    h_prev = out_hist[:, (s - 1) % S, :]

                    nc.tensor.matmul(
                        psum_rz[:, s, :],
                        whhT[:, 0 : 2 * NH],
                        h_prev,
                        start=False,
                        stop=True,
                        skip_group_check=True,
                    )
                    rzs = work.tile([2 * NH, BLOC], F32, tag="rzs")
                    nc.scalar.activation(
                        rzs, psum_rz[:, s, :], AF.Sigmoid, bias=bias_rz[:]
                    )

                    psum_nh = psum_nh_pool.tile([2 * NH, BLOC], F32, tag="nh")
                    nc.tensor.matmul(
                        psum_nh[NH : 2 * NH, :],
                        whhT[:, 2 * NH : 3 * NH],
                        h_prev,
                        start=True,
                        stop=True,
                        tile_position=(0, NH),
                    )
                    m_t = work.tile([2 * NH, BLOC], F32, tag="m")
                    nc.vector.scalar_tensor_tensor(
                        m_t[NH : 2 * NH, :],
                        psum_nh[NH : 2 * NH, :],
                        b_hhn[NH : 2 * NH, :],
                        rzs[NH : 2 * NH, :],
                        OP.add,
                        OP.mult,
                    )
                    nc.tensor.matmul(
                        psum_gn[:, s, :],
                        ident[NH : 2 * NH, :],
                        m_t[NH : 2 * NH, :],
                        start=False,
                        stop=True,
                        skip_group_check=True,
                        tile_position=(NH, 0),
                    )
                    n_t = work.tile([NH, BLOC], F32, tag="n")
                    nc.scalar.activation(
                        n_t, psum_gn[:, s, :], AF.Tanh, bias=b_ihn[:]
                    )

                    p_t = work.tile([NH, BLOC], F32, tag="p")
                    nc.vector.tensor_mul(p_t, rzs[0:NH, :], h_prev)
                    q_t = work.tile([NH, BLOC], F32, tag="q")
                    nc.vector.scalar_tensor_tensor(
                        q_t, rzs[0:NH, :], 1.0, n_t, OP.subtract, OP.mult
                    )
                    nc.vector.tensor_sub(out_hist[:, s, :], p_t, q_t)

                # ---- stream the chunk's hidden states out ----
                nc.sync.dma_start(
                    out=out_d[:, bass.ds(q * QLEN + iv, S), :], in_=out_hist[:]
                )

    _split_excess_waits(nc)
    return nc


_NC_CACHE = []


def _get_nc():
    if not _NC_CACHE:
        _NC_CACHE.append(_build())
    return _NC_CACHE[0]


def kernel(noise, w_ih, w_hh, b_ih, b_hh):
    noise = np.ascontiguousarray(np.asarray(noise, dtype=np.float32))
    w_ih = np.asarray(w_ih, dtype=np.float32)
    w_hh = np.asarray(w_hh, dtype=np.float32)
    b_ih = np.asarray(b_ih, dtype=np.float32)
    b_hh = np.asarray(b_hh, dtype=np.float32)

    # gate order on-chip is [z | r | n] so that z shares base partition 0
    # with h/n (blend ops) while r/nh live on partitions 64..127.
    zrn = np.concatenate([w_ih[NH : 2 * NH], w_ih[0:NH], w_ih[2 * NH :]], axis=0)
    zrn_hh = np.concatenate(
        [w_hh[NH : 2 * NH], w_hh[0:NH], w_hh[2 * NH :]], axis=0
    )
    bias_sum = b_ih + b_hh
    bias_zr = np.concatenate(
        [bias_sum[NH : 2 * NH], bias_sum[0:NH]]
    ).reshape(2 * NH, 1)
    b_hhn_pad = np.zeros((2 * NH, 1), dtype=np.float32)
    b_hhn_pad[NH:, 0] = b_hh[2 * NH :]
    ident_pad = np.zeros((2 * NH, NH), dtype=np.float32)
    ident_pad[NH:] = np.eye(NH, dtype=np.float32)
    shared = {
        "w_ihT": np.ascontiguousarray(zrn.T),
        "w_hhT": np.ascontiguousarray(zrn_hh.T),
        "bias_rz": np.ascontiguousarray(bias_zr),
        "b_ihn": np.ascontiguousarray(b_ih[2 * NH :].reshape(NH, 1)),
        "b_hhn": b_hhn_pad,
        "ident": ident_pad,
    }
    in_maps = []
    for c in range(NCORES):
        shard = noise[c * BLOC : (c + 1) * BLOC]  # (32, T, NI)
        in_maps.append(
            {"noiseT": np.ascontiguousarray(shard.transpose(2, 1, 0)), **shared}
        )

    nc = _get_nc()
    res = run_bass_kernel_spmd(
        nc, in_maps, core_ids=list(range(NCORES)), trace=TRACE
    )
    _LAST_RESULTS["res"] = res

    out = np.empty((B, T, NH), dtype=np.float32)
    for c in range(NCORES):
        out[c * BLOC : (c + 1) * BLOC] = res.results[c]["outT"].transpose(2, 1, 0)
    return out
